# revision 15
# baseline (speedup 1.0000x reference)
# Multi-head attention (B=4, S=2048, D=1024, H=16, Hd=64) on 8 trn2 NeuronCores.
#
# Sharding: tensor-parallel over heads. Each core owns 2 heads (a 128-wide
# slice of the Q/K/V projections and of the out-proj contraction dim), sees the
# full x, and produces a partial output [8192, 1024]; the host sums the 8
# partials (the TP all-reduce) and adds bo.
#
# On-device dataflow is fully "transposed" so no big matrix ever needs an
# on-chip transpose:
#   x^T (pre-transposed on host)  --proj-->  Q^T, K^T  [128, S*B] (head dims on
#   partitions) and V^T -> V via small PE transposes.
#   S^T tile [k,q] = matmul(lhsT=K^T ktile, rhs=Q^T)          (K = hd = 64)
#   P^T = exp(S^T * scale)   (softmax max-subtraction skipped; scores are O(1))
#   ctx^T/denoms = matmul(lhsT=V_aug ktile [128, 65], rhs=P^T) accumulated over
#   k tiles in PSUM; V_aug column 0 is ones, so PSUM row 0 = softmax denoms.
#   normalize: recip(row0) -> gpsimd partition_broadcast -> tensor_mul,
#   then SBUF->SBUF DMA into the stacked ctx^T tile [128, 2048] per batch.
#   out partial = matmul(lhsT=ctx^T qtile [128,128], rhs=Wo slice) per batch.

import os

import numpy as np

B = 4
S = 2048
D = 1024
H = 16
HD = 64
NCORES = 8
HPC = H // NCORES          # 2 heads per core
CW = HPC * HD              # 128: per-core slice width
T = B * S                  # 8192 tokens
NKT = D // 128             # 8 k-tiles over D for the projections
NTT = S // 128             # 16 token-tiles per batch
SCALE = 1.0 / float(np.sqrt(HD))

_NC_CACHE = {}


def _dtypes(mode):
    import concourse.mybir as mybir

    f32 = mybir.dt.float32
    f32r = mybir.dt.float32r
    bf16 = mybir.dt.bfloat16
    # st_xw: x^T + weights (DRAM + SBUF); st_qkv: Q^T/K^T/V/P^T tiles;
    # st_c: normalized ctx^T (out-proj lhsT); st_tr: V-transpose chain
    # (vtmp/identity/transpose-psum). walrus requires every producer of an
    # f32r matmul operand to emit dtype f32r, so the storage dtype carries it.
    if mode == "f32":
        return dict(st_xw=f32, st_qkv=f32, st_c=f32, st_tr=f32)
    if mode == "f32r":
        return dict(st_xw=f32r, st_qkv=f32r, st_c=f32r, st_tr=f32)
    if mode == "mixed":
        return dict(st_xw=f32r, st_qkv=bf16, st_c=f32r, st_tr=bf16)
    if mode == "bf16":
        return dict(st_xw=bf16, st_qkv=bf16, st_c=bf16, st_tr=bf16)
    raise ValueError(mode)


def build_nc(mode):
    import concourse.mybir as mybir
    from concourse import bacc
    from concourse.masks import make_identity
    from concourse.tile import TileContext

    dts = _dtypes(mode)
    f32 = mybir.dt.float32
    st_xw, st_qkv, st_c, st_tr = (
        dts["st_xw"], dts["st_qkv"], dts["st_c"], dts["st_tr"],
    )

    nc = bacc.Bacc()

    xT = nc.dram_tensor("xT", [D, T], st_xw, kind="ExternalInput")
    wq = nc.dram_tensor("wq", [D, CW], st_xw, kind="ExternalInput")
    wk = nc.dram_tensor("wk", [D, CW], st_xw, kind="ExternalInput")
    wv = nc.dram_tensor("wv", [D, CW], st_xw, kind="ExternalInput")
    wo = nc.dram_tensor("wo", [CW, D], st_xw, kind="ExternalInput")
    bq = nc.dram_tensor("bq", [CW], f32, kind="ExternalInput")
    bk = nc.dram_tensor("bk", [CW], f32, kind="ExternalInput")
    bv = nc.dram_tensor("bv", [CW], f32, kind="ExternalInput")
    out = nc.dram_tensor("out", [T, D], f32, kind="ExternalOutput")

    xT_v = xT.rearrange("(k p) t -> p k t", p=128)   # [128, NKT, T]
    wq_v = wq.rearrange("(k p) n -> p k n", p=128)   # [128, NKT, CW]
    wk_v = wk.rearrange("(k p) n -> p k n", p=128)
    wv_v = wv.rearrange("(k p) n -> p k n", p=128)

    TCH = 512                 # projection token-chunk
    NCH = S // TCH            # chunks per batch

    with TileContext(nc) as tc:
        with (
            tc.tile_pool(name="consts", bufs=1) as consts,
            tc.tile_pool(name="weights", bufs=1) as wpool,
            tc.tile_pool(name="qk", bufs=2) as qkpool,
            tc.tile_pool(name="vpool", bufs=2) as vpool,
            tc.tile_pool(name="xt", bufs=2) as xtpool,
            tc.tile_pool(name="vtmp", bufs=2) as vtmppool,
            tc.tile_pool(name="pt", bufs=2) as ptpool,
            tc.tile_pool(name="drain", bufs=2) as drainpool,
            tc.tile_pool(name="ctxt", bufs=2) as ctxtpool,
            tc.tile_pool(name="ostage", bufs=3) as opool,
            tc.tile_pool(name="ps_s", bufs=2, space="PSUM") as ps_s,
            tc.tile_pool(name="ps_ctx", bufs=3, space="PSUM") as ps_ctx,
            tc.tile_pool(name="ps_p", bufs=1, space="PSUM") as ps_p,
            tc.tile_pool(name="scr", bufs=4, space="DRAM") as scrpool,
        ):
            # ---- constants / weights ----
            ident = consts.tile([128, 128], st_tr, tag="ident")
            make_identity(nc, ident[:])
            zbias = consts.tile([128, 1], f32, tag="zbias")
            nc.vector.memset(zbias[:], 0.0)
            ones_col = consts.tile([128, 1], f32, tag="ones_col")
            nc.vector.memset(ones_col[:], 1.0)

            wq_sb = wpool.tile([128, NKT, CW], st_xw, tag="wq")
            wk_sb = wpool.tile([128, NKT, CW], st_xw, tag="wk")
            wv_sb = wpool.tile([128, NKT, CW], st_xw, tag="wv")
            wo_sb = wpool.tile([128, D], st_xw, tag="wo")
            nc.sync.dma_start(wq_sb[:], wq_v)
            nc.sync.dma_start(wk_sb[:], wk_v)
            nc.sync.dma_start(wv_sb[:], wv_v)
            nc.sync.dma_start(wo_sb[:], wo[:, :])
            bq_sb = wpool.tile([128, 1], f32, tag="bq")
            bk_sb = wpool.tile([128, 1], f32, tag="bk")
            bv_sb = wpool.tile([128, 1], f32, tag="bv")
            nc.sync.dma_start(bq_sb[:], bq[:, None])
            nc.sync.dma_start(bk_sb[:], bk[:, None])
            nc.sync.dma_start(bv_sb[:], bv[:, None])

            for b in range(B):
                boff = b * S
                # ---- projections for batch b: Q^T, K^T [128, S]; V tiles ----
                qTb = qkpool.tile([128, S], st_qkv, tag="qT")
                kTb = qkpool.tile([128, S], st_qkv, tag="kT")
                # V_aug per (head, token-tile): [128 tokens, 65] (col 0 = ones)
                vb = vpool.tile([128, HPC * NTT * 65], st_qkv, tag="v")
                vb3 = vb[:].rearrange("p (n c) -> p n c", c=65)
                nc.vector.tensor_copy(
                    vb3[:, :, 64:65], ones_col[:].to_broadcast((128, HPC * NTT, 1))
                )

                for ch in range(NCH):
                    toff = ch * TCH
                    xt = xtpool.tile([128, NKT, TCH], st_xw, tag="xt")
                    nc.sync.dma_start(xt[:], xT_v[:, :, boff + toff : boff + toff + TCH])

                    for w_sb, b_sb, dest in (
                        (wq_sb, bq_sb, qTb),
                        (wk_sb, bk_sb, kTb),
                    ):
                        ps = ps_p.tile([128, TCH], f32, tag="p")
                        for kt in range(NKT):
                            nc.tensor.matmul(
                                ps[:],
                                lhsT=w_sb[:, kt, :],
                                rhs=xt[:, kt, :],
                                start=(kt == 0),
                                stop=(kt == NKT - 1),
                            )
                        nc.vector.tensor_scalar_add(
                            dest[:, toff : toff + TCH], ps[:], b_sb[:]
                        )

                    psv = ps_p.tile([128, TCH], f32, tag="p")
                    for kt in range(NKT):
                        nc.tensor.matmul(
                            psv[:],
                            lhsT=wv_sb[:, kt, :],
                            rhs=xt[:, kt, :],
                            start=(kt == 0),
                            stop=(kt == NKT - 1),
                        )
                    vtmp = vtmppool.tile([128, TCH], st_tr, tag="vtmp")
                    nc.vector.tensor_scalar_add(vtmp[:], psv[:], bv_sb[:])
                    # V^T chunk -> V via PE transpose of 128x128 tiles
                    for tt in range(TCH // 128):
                        pst = ps_p.tile([128, 128], st_tr, tag="p")
                        nc.tensor.transpose(
                            pst[:], vtmp[:, tt * 128 : (tt + 1) * 128], ident[:]
                        )
                        gtt = ch * (TCH // 128) + tt  # token-tile id in batch
                        for h in range(HPC):
                            nc.vector.tensor_copy(
                                vb3[:, h * NTT + gtt, 0:64],
                                pst[:, h * 64 : (h + 1) * 64],
                            )

                # ---- attention for the two heads of batch b ----
                ctxTb = ctxtpool.tile([128, S], st_c, tag="ctxT")
                for h in range(HPC):
                    hsl = slice(h * 64, (h + 1) * 64)
                    for qc in range(2):
                        qoff = qc * 1024
                        cps = [
                            ps_ctx.tile([65, 512], f32, tag="ctx", name=f"ctxps_{b}_{h}_{qc}_{i}")
                            for i in range(2)
                        ]
                        for kt in range(NTT):
                            sps = ps_s.tile([128, 1024], f32, tag="s")
                            for half in range(2):
                                nc.tensor.matmul(
                                    sps[:, half * 512 : (half + 1) * 512],
                                    lhsT=kTb[hsl, kt * 128 : (kt + 1) * 128],
                                    rhs=qTb[hsl, qoff + half * 512 : qoff + (half + 1) * 512],
                                    start=True,
                                    stop=True,
                                )
                            pt = ptpool.tile([128, 1024], st_qkv, tag="pt")
                            nc.scalar.activation(
                                pt[:],
                                sps[:],
                                mybir.ActivationFunctionType.Exp,
                                bias=zbias[:],
                                scale=SCALE,
                            )
                            for half in range(2):
                                nc.tensor.matmul(
                                    cps[half][:],
                                    lhsT=vb3[:, h * NTT + kt, :],
                                    rhs=pt[:, half * 512 : (half + 1) * 512],
                                    start=(kt == 0),
                                    stop=(kt == NTT - 1),
                                )
                        # normalize and place into ctxTb rows [h*64, h*64+64)
                        for half in range(2):
                            # denom is PSUM row 64; recip it (lane 64), bounce
                            # through DRAM, DMA back partition-broadcast.
                            recip = drainpool.tile([65, 512], f32, tag="recip")
                            nc.vector.reciprocal(recip[64:65, :], cps[half][64:65, :])
                            scr = scrpool.tile([1, 512], f32, tag="scr")
                            nc.sync.dma_start(scr[0:1, :], recip[64:65, :])
                            rbc = drainpool.tile([64, 512], f32, tag="rbc")
                            nc.sync.dma_start(
                                rbc[:], scr[0:1, :].to_broadcast((64, 512))
                            )
                            stg = drainpool.tile([64, 512], st_c, tag="stg")
                            nc.vector.tensor_mul(
                                stg[:], cps[half][0:64, :], rbc[:]
                            )
                            nc.sync.dma_start(
                                ctxTb[
                                    h * 64 : (h + 1) * 64,
                                    qoff + half * 512 : qoff + (half + 1) * 512,
                                ],
                                stg[:],
                            )

                # ---- out-projection partial for batch b ----
                for qt in range(NTT):
                    for nch in range(2):
                        pso = ps_s.tile([128, 1024], f32, tag="s")
                        nc.tensor.matmul(
                            pso[:, 0:512],
                            lhsT=ctxTb[:, qt * 128 : (qt + 1) * 128],
                            rhs=wo_sb[:, nch * 512 : (nch + 1) * 512],
                            start=True,
                            stop=True,
                        )
                        ost = opool.tile([128, 512], f32, tag="ost")
                        nc.vector.tensor_copy(ost[:], pso[:, 0:512])
                        nc.sync.dma_start(
                            out[
                                boff + qt * 128 : boff + (qt + 1) * 128,
                                nch * 512 : (nch + 1) * 512,
                            ],
                            ost[:],
                        )
    nc.compile()
    return nc


def _np_dtype(mode):
    if mode == "bf16":
        import ml_dtypes

        return np.dtype(ml_dtypes.bfloat16)
    return np.dtype(np.float32)


def make_in_maps(x, Wq, bq, Wk, bk, Wv, bv, Wo, bo, mode):
    ndt = _np_dtype(mode)
    xT = np.ascontiguousarray(
        np.asarray(x, dtype=np.float32).reshape(T, D).T
    ).astype(ndt)
    Wq = np.asarray(Wq, np.float32)
    Wk = np.asarray(Wk, np.float32)
    Wv = np.asarray(Wv, np.float32)
    Wo = np.asarray(Wo, np.float32)
    in_maps = []
    for c in range(NCORES):
        sl = slice(c * CW, (c + 1) * CW)
        in_maps.append(
            {
                "xT": xT,
                "wq": np.ascontiguousarray(Wq[:, sl]).astype(ndt),
                "wk": np.ascontiguousarray(Wk[:, sl]).astype(ndt),
                "wv": np.ascontiguousarray(Wv[:, sl]).astype(ndt),
                "wo": np.ascontiguousarray(Wo[sl, :]).astype(ndt),
                "bq": np.ascontiguousarray(np.asarray(bq, np.float32)[sl]),
                "bk": np.ascontiguousarray(np.asarray(bk, np.float32)[sl]),
                "bv": np.ascontiguousarray(np.asarray(bv, np.float32)[sl]),
            }
        )
    return in_maps


def _ensure_ntff_hook():
    """The agent image's antenv lacks axon_hooks; synthesize it so
    run_bass_kernel_spmd(trace=True) can capture NTFF profiles."""
    try:
        from antenv.axon_hooks import get_axon_ntff_profile_hook  # noqa: F401

        return
    except ImportError:
        pass
    import sys
    import types

    try:
        if "/root/.axon_site" not in sys.path:
            sys.path.insert(0, "/root/.axon_site")
        from trn_agent_boot.trn_boot import _ntff_profile_via_ctypes

        hook = _ntff_profile_via_ctypes("/opt/axon/libaxon_pjrt.so")
    except Exception as e:  # no axon / no so — tracing just degrades
        print("ntff hook shim failed:", e)
        return
    import antenv

    m = types.ModuleType("antenv.axon_hooks")
    m._hook = hook
    m.set_axon_ntff_profile_hook = lambda h: setattr(m, "_hook", h)
    m.get_axon_ntff_profile_hook = lambda: m._hook
    sys.modules["antenv.axon_hooks"] = m
    antenv.axon_hooks = m


def run(x, Wq, bq, Wk, bk, Wv, bv, Wo, bo, trace=False, mode=None):
    from concourse.bass_utils import run_bass_kernel_spmd

    if trace:
        _ensure_ntff_hook()

    if mode is None:
        mode = os.environ.get("MHA_MODE", "f32r")
    if mode not in _NC_CACHE:
        _NC_CACHE[mode] = build_nc(mode)
    nc = _NC_CACHE[mode]
    in_maps = make_in_maps(x, Wq, bq, Wk, bk, Wv, bv, Wo, bo, mode)
    res = run_bass_kernel_spmd(nc, in_maps, core_ids=list(range(NCORES)), trace=trace)
    acc = np.zeros((T, D), dtype=np.float64)
    for r in res.results:
        acc += r["out"].astype(np.float64)
    o = (acc + np.asarray(bo, np.float64)[None, :]).astype(np.float32)
    return o.reshape(B, S, D), res


def kernel(x, Wq, bq, Wk, bk, Wv, bv, Wo, bo):
    o, _ = run(x, Wq, bq, Wk, bk, Wv, bv, Wo, bo, trace=False)
    return o
